# revision 10
# baseline (speedup 1.0000x reference)
"""AdaCare kernel for 8 TRN2 NeuronCores — data-parallel over batch.

Each core handles 16 of 128 samples. All parameters replicated.
Layout on device: channels on partitions, positions (b*64+v, b-major) on
the free dimension. All matmuls in bf16 with f32 PSUM accumulation.
"""
import sys
sys.path.insert(0, '/opt/trn_rl_repo')

import numpy as np
import ml_dtypes

import concourse.bass as bass
import concourse.bacc as bacc
import concourse.mybir as mybir
import concourse.tile as tile
from concourse.masks import make_identity
from concourse.bass_utils import run_bass_kernel_spmd

BF16 = mybir.dt.bfloat16
F32 = mybir.dt.float32
I32 = mybir.dt.int32
AF = mybir.ActivationFunctionType
OP = mybir.AluOpType

# problem shapes (hardcoded per spec)
B, V, N, D = 128, 64, 32, 128
C_IN = N * D            # 4096
CC = 192                # 3 * 64 conv channels
H = 256
OUTD = 2
NUM_NODES = 10000
R_C = 48
R_V = 1024
NCORES = 8
BL = B // NCORES        # 16 samples per core
POS = BL * V            # 1024 positions per core
NK = C_IN // 128        # 32 channel tiles
DILS = (1, 3, 5)

_cache = {}


def _build():
    nc = bacc.Bacc("TRN2", target_bir_lowering=False, debug=False,
                   num_devices=NCORES, num_swdge_queues=4)
    P = lambda name, shape, dt: nc.declare_dram_parameter(name, list(shape), dt, isOutput=False)

    idx_d = P("idx", (128, N * 64), mybir.dt.int16)
    emb_d = P("emb", (NUM_NODES + 1, D), BF16)
    wall_d = P("wall", (C_IN, 384), BF16)       # conv taps, transposed
    convb_d = P("convb", (128, 2), F32)
    wcin_d = P("wcin", (C_IN, R_V), BF16)
    bcin_d = P("bcin", (128, 8), F32)
    wrin_d = P("wrin", (R_V, C_IN), BF16)
    brin_d = P("brin", (128, 32), F32)
    wcc_d = P("wcc", (CC, R_C), BF16)
    bcc_d = P("bcc", (R_C, 1), F32)
    wrc_d = P("wrc", (R_C, CC), BF16)
    brc_d = P("brc", (128, 2), F32)
    wih_d = P("wih", (C_IN + CC, 768), BF16)    # rows: [in_se 4096; conv_se 192]
    biasg_d = P("biasg", (128, 6), F32)
    whh_d = P("whh", (H, 768), BF16)
    bhhn_d = P("bhhn", (128, 32), BF16)
    outw_d = P("outw", (H, OUTD), BF16)
    outb_d = P("outb", (OUTD, 1), F32)
    invt_d = P("invt", (128, POS), F32)
    out_d = nc.declare_dram_parameter("out", [OUTD, POS], F32, isOutput=True)

    with tile.TileContext(nc) as tc:
        with tc.tile_pool(name="const", bufs=1) as const, \
             tc.tile_pool(name="xp", bufs=1) as xp, \
             tc.tile_pool(name="gst", bufs=8) as gst, \
             tc.tile_pool(name="wst", bufs=4) as wst, \
             tc.tile_pool(name="wihp", bufs=3) as wihp, \
             tc.tile_pool(name="mid", bufs=1) as mid, \
             tc.tile_pool(name="wk", bufs=3) as wk, \
             tc.tile_pool(name="gru", bufs=4) as gru:

            # ---- constants / small weights -------------------------------
            idx_sb = const.tile([128, N * 64], mybir.dt.int16)
            nc.gpsimd.dma_start(idx_sb[:], idx_d[:])
            invt = const.tile([128, POS], F32)
            nc.sync.dma_start(invt[:], invt_d[:])
            ident = const.tile([128, 128], BF16)
            make_identity(nc, ident[:])
            convb = const.tile([128, 2], F32)
            nc.sync.dma_start(convb[:], convb_d[:])
            bcin = const.tile([128, 8], F32)
            nc.sync.dma_start(bcin[:], bcin_d[:])
            brin = const.tile([128, 32], F32)
            nc.sync.dma_start(brin[:], brin_d[:])
            wcc0 = const.tile([128, R_C], BF16)
            nc.sync.dma_start(wcc0[:], wcc_d[0:128, :])
            wcc1 = const.tile([64, R_C], BF16)
            nc.sync.dma_start(wcc1[:], wcc_d[128:192, :])
            bcc = const.tile([R_C, 1], F32)
            nc.sync.dma_start(bcc[:], bcc_d[:])
            wrc = const.tile([R_C, CC], BF16)
            nc.sync.dma_start(wrc[:], wrc_d[:])
            brc = const.tile([128, 2], F32)
            nc.sync.dma_start(brc[:], brc_d[:])
            biasg = const.tile([128, 6], F32)
            nc.sync.dma_start(biasg[:], biasg_d[:])
            whh0 = const.tile([128, 768], BF16)
            nc.sync.dma_start(whh0[:], whh_d[0:128, :])
            whh1 = const.tile([128, 768], BF16)
            nc.sync.dma_start(whh1[:], whh_d[128:256, :])
            bhhn = const.tile([128, 32], BF16)
            nc.sync.dma_start(bhhn[:], bhhn_d[:])
            outw0 = const.tile([128, OUTD], BF16)
            nc.sync.dma_start(outw0[:], outw_d[0:128, :])
            outw1 = const.tile([128, OUTD], BF16)
            nc.sync.dma_start(outw1[:], outw_d[128:256, :])
            outb = const.tile([OUTD, 1], F32)
            nc.sync.dma_start(outb[:], outb_d[:])

            # ---- embedding gather into (chan, pos) layout ----------------
            x_sb = [xp.tile([128, POS], BF16, tag=f"x{n}", name=f"x_sb{n}")
                    for n in range(N)]
            for n in range(N):
                nc.gpsimd.dma_gather(
                    out_ap=x_sb[n][:]
                    .rearrange("p (one i) -> p one i", one=1),
                    in_ap=emb_d[:, :],
                    idxs_ap=idx_sb[:, n * 64:(n + 1) * 64],
                    num_idxs=POS,
                    num_idxs_reg=POS,
                    elem_size=D,
                    transpose=True,
                    single_packet=False,
                    queue_num=0,
                )

            # persistent mid tensors
            s_bf = [mid.tile([128, POS], BF16, tag=f"s{m}", name=f"s_bf{m}")
                    for m in range(8)]
            gi_sb = [mid.tile([128, POS], BF16, tag=f"gi{m}", name=f"gi_sb{m}")
                     for m in range(6)]
            hs0 = mid.tile([128, POS], BF16, tag="hs0")   # col t*16+b
            hs1 = mid.tile([128, POS], BF16, tag="hs1")

            # ---- streaming phase: conv (both chunks) + P1 m0, paced by gathers
            with tc.tile_pool(name="cv", bufs=1) as cvp:
                wallst = [wst.tile([128, NK * 128], BF16, tag="w",
                                   name=f"wallst{m}") for m in range(3)]
                for m in range(3):
                    nc.sync.dma_start(
                        wallst[m][:].rearrange("p (kt m) -> p kt m", m=128),
                        wall_d[:, m * 128:(m + 1) * 128]
                        .rearrange("(kt p) m -> p kt m", p=128))
                wcin0 = wst.tile([128, NK * 128], BF16, tag="w", name="wcin0st")
                nc.sync.dma_start(
                    wcin0[:].rearrange("p (kt m) -> p kt m", m=128),
                    wcin_d[:, 0:128].rearrange("(kt p) m -> p kt m", p=128))
                conv_bf = {}
                conv_se = {}
                with tc.tile_pool(name="psS", bufs=1, space="PSUM") as psS:
                    g_ps = [[psS.tile([128, 512], F32, tag=f"g{m}{ch}",
                                      name=f"g_ps{m}{ch}") for ch in range(2)]
                            for m in range(3)]
                    p10 = [psS.tile([128, 512], F32, tag=f"p10{ch}",
                                    name=f"p10{ch}") for ch in range(2)]
                    for k in range(NK):
                        for ch in range(2):
                            c0 = ch * 512
                            for m in range(3):
                                nc.tensor.matmul(
                                    g_ps[m][ch][:],
                                    wallst[m][:, k * 128:(k + 1) * 128],
                                    x_sb[k][:, c0:c0 + 512],
                                    start=(k == 0), stop=(k == NK - 1))
                            nc.tensor.matmul(
                                p10[ch][:],
                                wcin0[:, k * 128:(k + 1) * 128],
                                x_sb[k][:, c0:c0 + 512],
                                start=(k == 0), stop=(k == NK - 1))
                    for ch in range(2):
                        c0 = ch * 512
                        CS = slice(c0, c0 + 512)
                        # conv tap combine: rows 0:64 = tap0 (x[t-d]), 64:128 = tap1
                        tmpc = [cvp.tile([64, 512], F32, tag=f"tc{m}",
                                         name=f"tmpc{m}_{ch}") for m in range(3)]
                        for m, dil in enumerate(DILS):
                            nc.scalar.activation(tmpc[m][:], g_ps[m][ch][64:128, :],
                                                 AF.Copy)
                            o3 = tmpc[m][:].rearrange("p (b v) -> p b v", v=64)
                            i3 = g_ps[m][ch][0:64, :].rearrange("p (b v) -> p b v", v=64)
                            nc.vector.tensor_tensor(
                                out=o3[:, :, dil:64], in0=o3[:, :, dil:64],
                                in1=i3[:, :, 0:64 - dil], op=OP.add)
                        cb0 = cvp.tile([128, 512], BF16, tag=f"cb0{ch}",
                                       name=f"cb0_{ch}")
                        cb1 = cvp.tile([64, 512], BF16, tag=f"cb1{ch}",
                                       name=f"cb1_{ch}")
                        nc.scalar.activation(cb0[0:64, :], tmpc[0][:], AF.Relu,
                                             bias=convb[0:64, 0:1])
                        nc.scalar.activation(cb0[64:128, :], tmpc[1][:], AF.Relu,
                                             bias=convb[64:128, 0:1])
                        nc.scalar.activation(cb1[:], tmpc[2][:], AF.Relu,
                                             bias=convb[0:64, 1:2])
                        conv_bf[ch] = (cb0, cb1)
                        # P1 m=0 epilogue
                        cm = wk.tile([128, 512], F32, tag="cm", name=f"cm0_{ch}")
                        for b in range(8):
                            nc.vector.tensor_tensor_scan(
                                out=cm[:, b * 64:(b + 1) * 64],
                                data0=p10[ch][:, b * 64:(b + 1) * 64],
                                data1=invt[:, 0:64],
                                initial=0.0, op0=OP.add, op1=OP.bypass)
                        nc.vector.tensor_tensor(out=cm[:], in0=cm[:],
                                                in1=invt[:, CS], op=OP.mult)
                        nc.scalar.activation(s_bf[0][:, CS], cm[:], AF.Relu,
                                             bias=bcin[:, 0:1])

                for ch in range(2):
                    c0 = ch * 512
                    CS = slice(c0, c0 + 512)
                    conv_bf0, conv_bf1 = conv_bf[ch]
                    # ---- phase C: conv squeeze-excite ---------------------
                    conv_se0 = cvp.tile([128, 512], BF16, tag=f"cse0{ch}",
                                        name=f"cse0_{ch}")
                    conv_se1 = cvp.tile([64, 512], BF16, tag=f"cse1{ch}",
                                        name=f"cse1_{ch}")
                    conv_se[ch] = (conv_se0, conv_se1)
                    with tc.tile_pool(name=f"psC{ch}", bufs=1, space="PSUM") as psC:
                        sc_ps = psC.tile([R_C, 512], F32, tag="scp")
                        nc.tensor.matmul(sc_ps[:], wcc0[:], conv_bf0[:],
                                         start=True, stop=False)
                        nc.tensor.matmul(sc_ps[:], wcc1[:], conv_bf1[:],
                                         start=False, stop=True)
                        cmc = cvp.tile([R_C, 512], F32, tag="cmc", name=f"cmc{ch}")
                        for b in range(8):
                            nc.vector.tensor_tensor_scan(
                                out=cmc[:, b * 64:(b + 1) * 64],
                                data0=sc_ps[:, b * 64:(b + 1) * 64],
                                data1=invt[0:R_C, 0:64],
                                initial=0.0, op0=OP.add, op1=OP.bypass)
                        nc.vector.tensor_tensor(out=cmc[:], in0=cmc[:],
                                                in1=invt[0:R_C, CS], op=OP.mult)
                        se_bf = cvp.tile([R_C, 512], BF16, tag="sebf",
                                         name=f"sebf{ch}")
                        nc.scalar.activation(se_bf[:], cmc[:], AF.Relu, bias=bcc[:])
                        scg0 = psC.tile([128, 512], F32, tag="scg0")
                        scg1 = psC.tile([64, 512], F32, tag="scg1")
                        nc.tensor.matmul(scg0[:], wrc[:, 0:128], se_bf[:],
                                         start=True, stop=True)
                        nc.tensor.matmul(scg1[:], wrc[:, 128:192], se_bf[:],
                                         start=True, stop=True)
                        sg0 = cvp.tile([128, 512], BF16, tag="sg0", name=f"sg0{ch}")
                        sg1 = cvp.tile([64, 512], BF16, tag="sg1", name=f"sg1{ch}")
                        nc.scalar.activation(sg0[:], scg0[:], AF.Sigmoid,
                                             bias=brc[:, 0:1])
                        nc.scalar.activation(sg1[:], scg1[:], AF.Sigmoid,
                                             bias=brc[0:64, 1:2])
                        nc.vector.tensor_tensor(out=conv_se0[:], in0=conv_bf0[:],
                                                in1=sg0[:], op=OP.mult)
                        nc.vector.tensor_tensor(out=conv_se1[:], in0=conv_bf1[:],
                                                in1=sg1[:], op=OP.mult)

                    # ---- phase D1: P1 m=1..7 ------------------------------
                    with tc.tile_pool(name=f"psP{ch}", bufs=2, space="PSUM") as psP:
                        for m in range(1, 8):
                            wstrip = wst.tile([128, NK * 128], BF16, tag="w",
                                              name="wstrip")
                            nc.sync.dma_start(
                                wstrip[:].rearrange("p (kt m) -> p kt m", m=128),
                                wcin_d[:, m * 128:(m + 1) * 128]
                                .rearrange("(kt p) m -> p kt m", p=128))
                            p1 = psP.tile([128, 512], F32, tag="p1")
                            for k in range(NK):
                                nc.tensor.matmul(
                                    p1[:],
                                    wstrip[:, k * 128:(k + 1) * 128],
                                    x_sb[k][:, c0:c0 + 512],
                                    start=(k == 0), stop=(k == NK - 1))
                            cm = wk.tile([128, 512], F32, tag="cm")
                            for b in range(8):
                                nc.vector.tensor_tensor_scan(
                                    out=cm[:, b * 64:(b + 1) * 64],
                                    data0=p1[:, b * 64:(b + 1) * 64],
                                    data1=invt[:, 0:64],
                                    initial=0.0, op0=OP.add, op1=OP.bypass)
                            nc.vector.tensor_tensor(out=cm[:], in0=cm[:],
                                                    in1=invt[:, CS], op=OP.mult)
                            nc.scalar.activation(s_bf[m][:, CS], cm[:], AF.Relu,
                                                 bias=bcin[:, m:m + 1])

                    # ---- phase D2: S2 -> sigmoid -> in_se -> gi -----------
                    conv_se0, conv_se1 = conv_se[ch]
                    with tc.tile_pool(name=f"psD{ch}", bufs=1, space="PSUM") as psD:
                        gi_ps = [psD.tile([128, 512], F32, tag=f"gip{m}",
                                          name=f"gi_ps{m}") for m in range(6)]
                        for m in range(NK):
                            wstrip = wst.tile([128, NK * 128], BF16, tag="w",
                                              name="wstrip")
                            nc.sync.dma_start(
                                wstrip[:, 0:R_V].rearrange("p (kt m) -> p kt m", m=128),
                                wrin_d[:, m * 128:(m + 1) * 128]
                                .rearrange("(kt p) m -> p kt m", p=128))
                            wihr = wihp.tile([128, 768], BF16, tag="wihr")
                            nc.sync.dma_start(
                                wihr[:], wih_d[m * 128:(m + 1) * 128, :])
                            s2 = psD.tile([128, 512], F32, tag="s2", bufs=2)
                            for k in range(8):
                                nc.tensor.matmul(
                                    s2[:],
                                    wstrip[:, k * 128:(k + 1) * 128],
                                    s_bf[k][:, CS],
                                    start=(k == 0), stop=(k == 7))
                            sg = wk.tile([128, 512], BF16, tag="sg")
                            nc.scalar.activation(sg[:], s2[:], AF.Sigmoid,
                                                 bias=brin[:, m:m + 1])
                            ins = wk.tile([128, 512], BF16, tag="ins")
                            nc.vector.tensor_tensor(
                                out=ins[:],
                                in0=x_sb[m][:, c0:c0 + 512],
                                in1=sg[:], op=OP.mult)
                            for mg in range(6):
                                nc.tensor.matmul(
                                    gi_ps[mg][:],
                                    wihr[:, mg * 128:(mg + 1) * 128],
                                    ins[:],
                                    start=(m == 0), stop=False)
                        # conv_se contribution (wih rows 4096:4288)
                        wihc0 = wihp.tile([128, 768], BF16, tag="wihr")
                        nc.sync.dma_start(wihc0[:], wih_d[4096:4224, :])
                        wihc1 = wihp.tile([64, 768], BF16, tag="wihc1")
                        nc.sync.dma_start(wihc1[:], wih_d[4224:4288, :])
                        for mg in range(6):
                            nc.tensor.matmul(
                                gi_ps[mg][:], wihc0[:, mg * 128:(mg + 1) * 128],
                                conv_se0[:], start=False, stop=False)
                            nc.tensor.matmul(
                                gi_ps[mg][:], wihc1[:, mg * 128:(mg + 1) * 128],
                                conv_se1[:], start=False, stop=True)
                            nc.scalar.activation(gi_sb[mg][:, CS], gi_ps[mg][:],
                                                 AF.Identity,
                                                 bias=biasg[:, mg:mg + 1])

            # ---- phase E: GRU over 64 steps ------------------------------
            with tc.tile_pool(name="psG", bufs=2, space="PSUM") as psG:
                for t in range(V):
                    rzp = psG.tile([128, 64], F32, tag="rz")
                    npp = psG.tile([128, 32], F32, tag="np")
                    tcol = slice(t, POS, 64)      # (128, 16) strided by sample
                    # preloads: only the FIRST matmul in the bank may start=True
                    for mg in range(4):
                        nc.tensor.matmul(rzp[:, mg * 16:(mg + 1) * 16],
                                         ident[:], gi_sb[mg][:, tcol],
                                         start=(mg == 0), stop=(t == 0))
                    for g in range(2):
                        nc.tensor.matmul(npp[:, g * 16:(g + 1) * 16],
                                         ident[:], bhhn[:, g * 16:(g + 1) * 16],
                                         start=(g == 0), stop=(t == 0))
                    if t > 0:
                        h0 = hs0[:, (t - 1) * 16: t * 16]
                        h1 = hs1[:, (t - 1) * 16: t * 16]
                        for mg in range(4):
                            nc.tensor.matmul(rzp[:, mg * 16:(mg + 1) * 16],
                                             whh0[:, mg * 128:(mg + 1) * 128],
                                             h0, start=False, stop=False)
                            nc.tensor.matmul(rzp[:, mg * 16:(mg + 1) * 16],
                                             whh1[:, mg * 128:(mg + 1) * 128],
                                             h1, start=False, stop=True)
                        for g in range(2):
                            mgw = 4 + g
                            nc.tensor.matmul(npp[:, g * 16:(g + 1) * 16],
                                             whh0[:, mgw * 128:(mgw + 1) * 128],
                                             h0, start=False, stop=False)
                            nc.tensor.matmul(npp[:, g * 16:(g + 1) * 16],
                                             whh1[:, mgw * 128:(mgw + 1) * 128],
                                             h1, start=False, stop=True)
                    r_bf = gru.tile([128, 32], BF16, tag="r")
                    nc.scalar.activation(r_bf[:], rzp[:, 0:32], AF.Sigmoid)
                    z_bf = gru.tile([128, 32], BF16, tag="z")
                    nc.scalar.activation(z_bf[:], rzp[:, 32:64], AF.Sigmoid)
                    zn_bf = gru.tile([128, 32], BF16, tag="zn")
                    nc.scalar.activation(zn_bf[:], rzp[:, 32:64], AF.Sigmoid,
                                         scale=-1.0)
                    tmp = gru.tile([128, 32], F32, tag="tmp")
                    nc.vector.tensor_tensor(out=tmp[:], in0=r_bf[:], in1=npp[:],
                                            op=OP.mult)
                    pn = gru.tile([128, 32], F32, tag="pn")
                    nc.vector.tensor_tensor(out=pn[:, 0:16], in0=tmp[:, 0:16],
                                            in1=gi_sb[4][:, tcol], op=OP.add)
                    nc.vector.tensor_tensor(out=pn[:, 16:32], in0=tmp[:, 16:32],
                                            in1=gi_sb[5][:, tcol], op=OP.add)
                    n_bf = gru.tile([128, 32], BF16, tag="n")
                    nc.scalar.activation(n_bf[:], pn[:], AF.Tanh)
                    nz = gru.tile([128, 32], F32, tag="nz")
                    nc.vector.tensor_tensor(out=nz[:], in0=n_bf[:], in1=zn_bf[:],
                                            op=OP.mult)
                    ts = slice(t * 16, (t + 1) * 16)
                    if t > 0:
                        zh = gru.tile([128, 32], F32, tag="zh")
                        nc.vector.tensor_tensor(out=zh[:, 0:16],
                                                in0=z_bf[:, 0:16],
                                                in1=hs0[:, (t - 1) * 16: t * 16],
                                                op=OP.mult)
                        nc.vector.tensor_tensor(out=zh[:, 16:32],
                                                in0=z_bf[:, 16:32],
                                                in1=hs1[:, (t - 1) * 16: t * 16],
                                                op=OP.mult)
                        nc.vector.tensor_tensor(out=hs0[:, ts], in0=nz[:, 0:16],
                                                in1=zh[:, 0:16], op=OP.add)
                        nc.vector.tensor_tensor(out=hs1[:, ts], in0=nz[:, 16:32],
                                                in1=zh[:, 16:32], op=OP.add)
                    else:
                        nc.vector.tensor_copy(hs0[:, ts], nz[:, 0:16])
                        nc.vector.tensor_copy(hs1[:, ts], nz[:, 16:32])

                # ---- phase F: output projection ---------------------------
                for ch in range(2):
                    CS = slice(ch * 512, ch * 512 + 512)
                    po = psG.tile([OUTD, 512], F32, tag="po")
                    nc.tensor.matmul(po[:], outw0[:], hs0[:, CS],
                                     start=True, stop=False)
                    nc.tensor.matmul(po[:], outw1[:], hs1[:, CS],
                                     start=False, stop=True)
                    ob = gru.tile([OUTD, 512], F32, tag="ob")
                    nc.scalar.activation(ob[:], po[:], AF.Identity, bias=outb[:])
                    nc.sync.dma_start(out_d[:, CS], ob[:])

    nc.finalize()
    return nc


def _bf16(a):
    return np.ascontiguousarray(a.astype(ml_dtypes.bfloat16))


def _prep(inputs):
    f = lambda k: np.asarray(inputs[k], dtype=np.float32)
    emb = _bf16(f('node_embed'))                                 # (10001,128)

    # conv taps stacked: cols [d1t0|d1t1|d3t0|d3t1|d5t0|d5t1], transposed
    taps = []
    for w in ('conv1_w', 'conv3_w', 'conv5_w'):
        cw = f(w)                                                # (64,4096,2)
        taps.append(cw[:, :, 0])
        taps.append(cw[:, :, 1])
    wall = _bf16(np.concatenate(taps, axis=0).T)                 # (4096,384)
    convb = np.zeros((128, 2), np.float32)
    convb[0:64, 0] = f('conv1_b')
    convb[64:128, 0] = f('conv3_b')
    convb[0:64, 1] = f('conv5_b')

    wcin = _bf16(f('inputse_c_w').T)                             # (4096,1024)
    bcin = np.ascontiguousarray(f('inputse_c_b').reshape(8, 128).T)
    wrin = _bf16(f('inputse_r_w').T)                             # (1024,4096)
    brin = np.ascontiguousarray(f('inputse_r_b').reshape(32, 128).T)
    wcc = _bf16(f('convse_c_w').T)                               # (192,48)
    bcc = np.ascontiguousarray(f('convse_c_b').reshape(R_C, 1))
    wrc = _bf16(f('convse_r_w').T)                               # (48,192)
    brc = np.zeros((128, 2), np.float32)
    brc[:, 0] = f('convse_r_b')[0:128]
    brc[0:64, 1] = f('convse_r_b')[128:192]

    w_ih = f('w_ih')                                             # (768, 4288)
    wih = _bf16(np.concatenate([w_ih[:, CC:].T, w_ih[:, 0:CC].T], axis=0))
    b_ih, b_hh = f('b_ih'), f('b_hh')
    biasg = np.zeros((128, 6), np.float32)
    for mg in range(4):
        biasg[:, mg] = b_ih[mg * 128:(mg + 1) * 128] + b_hh[mg * 128:(mg + 1) * 128]
    for mg in (4, 5):
        biasg[:, mg] = b_ih[mg * 128:(mg + 1) * 128]
    whh = _bf16(f('w_hh').T)                                     # (256,768)
    bhhn = np.zeros((128, 32), np.float32)
    bhhn[:, 0:16] = b_hh[512:640, None]
    bhhn[:, 16:32] = b_hh[640:768, None]
    bhhn = _bf16(bhhn)
    outw = _bf16(f('out_w').T)                                   # (256,2)
    outb = np.ascontiguousarray(f('out_b').reshape(OUTD, 1))

    invt = np.broadcast_to(
        np.tile(1.0 / np.arange(1, V + 1, dtype=np.float32), BL),
        (128, POS)).copy()

    shared = dict(emb=emb, wall=wall, convb=convb, wcin=wcin, bcin=bcin,
                  wrin=wrin, brin=brin, wcc=wcc, bcc=bcc, wrc=wrc, brc=brc,
                  wih=wih, biasg=biasg, whh=whh, bhhn=bhhn, outw=outw,
                  outb=outb, invt=invt)

    ids = np.asarray(inputs['node_ids']).astype(np.int16)        # (B,V,N)
    in_maps = []
    for c in range(NCORES):
        loc = ids[c * BL:(c + 1) * BL]                           # (16,64,32)
        # dma_gather idx layout: idx16[p, n*64+s] = flat_n[s*16 + p%16]
        flat = loc.reshape(POS, N)                               # [pos, n]
        arr = flat.reshape(64, 16, N)                            # [s, p16, n]
        blk = arr.transpose(1, 2, 0).reshape(16, N * 64)         # [p16, n*64+s]
        idx = np.ascontiguousarray(np.tile(blk, (8, 1)))         # (128, n*64+s)
        in_maps.append({'idx': idx, **shared})
    return in_maps


def kernel(**inputs):
    nc = _cache.get('nc')
    if nc is None:
        nc = _cache['nc'] = _build()
    in_maps = _prep(inputs)
    res = run_bass_kernel_spmd(nc, in_maps, core_ids=list(range(NCORES)))

    mask = np.asarray(inputs['attn_mask'])
    valid = ~mask.all(axis=-1)                                   # (B,V)
    last = valid.sum(axis=-1).astype(np.int64) - 1               # (B,)

    out = np.empty((B, OUTD), np.float32)
    for c in range(NCORES):
        o = np.asarray(res.results[c]['out']).reshape(OUTD, V, BL)  # [o,t,b]
        for b in range(BL):
            out[c * BL + b] = o[:, last[c * BL + b], b]
    return out
